# revision 3
# baseline (speedup 1.0000x reference)
"""Trainium2 Bass kernel for 16-head causal attention (transposed-softmax variant).

Problem shapes: x [8, 1024, 1024]; W_K/W_Q/W_V [16, 64, 1024]; W_O [1024, 1024].
Sharding: pure data-parallel over batch (8 batch elements -> 8 cores), weights
replicated, no collectives.

Per-core pipeline (one batch element, seq=1024, d_embed=1024, 16 heads x 64):
  1. QKV projections as K_T/Q_T [heads*64, seq] and V [seq, heads*64], fp16
     operands, fp32 PSUM accumulation. W_Q is pre-scaled by 1/sqrt(d_head) on
     the host so scores come out pre-scaled.
  2. Per head pair: scores S[c, C] = K[c].Q[C] for causal-allowed C-chunks
     only, the two heads' K=64 matmuls interleaved so they run concurrently in
     disjoint PE row-groups. No additive masking on the PE: the diagonal
     128-block is masked multiplicatively after exp (step 3).
  3. Softmax over C without max-subtraction (scores ~ N(0,1)): exp on the
     scalar engine (no accumulator), then ONE fused vector-engine
     tensor_tensor_reduce per (head, row-tile): E *= m1-slice (ones over the
     strict prefix, lower-triangular 0/1 over the diagonal block) in place,
     with accum_out = causal row sum. This keeps the scalar engine exp-only
     (no READ_ACCUMULATOR) and removes all mask matmuls from the PE.
     Reciprocals are batched per head ([128, 8] in one op); V rows are scaled
     by 1/rowsum on the gpsimd engine (otherwise idle).
  4. Z^T[h, C] += V'^T E per c-tile (descending i, N trimmed to the causal
     prefix, two heads col-split in one PSUM bank); output projection
     Z_flat @ W_O^T, PSUM->SBUF casts alternating vector/scalar engines.

Scheduling: the scalar engine carries ~110us of exp and the PE ~140us of
matmul; dense projection groups are spread as fillers across ALL eight pairs
(not front-loaded) so the PE never starves mid-kernel and the HAM clock gate
stays at full rate. Each pair's AV block interleaves into the next pair's
rows; kq-group weights for pair p+1 are emitted inside pair p.
"""

import numpy as np

S, E, A, H, B = 1024, 1024, 16, 64, 8
P = 128          # partitions

_cache = {}


def _off(i):
    """Compact E-buffer offset of row-tile i (valid width of row i is (i+1)*P)."""
    return P * i * (i + 1) // 2


EW = _off(8)     # 4608 columns total


def _build_nc():
    import concourse.bass as bass
    import concourse.mybir as mybir
    from concourse.tile import TileContext

    f16 = mybir.dt.float16
    f32 = mybir.dt.float32
    Exp = mybir.ActivationFunctionType.Exp
    mult = mybir.AluOpType.mult
    add = mybir.AluOpType.add

    nc = bass.Bass()
    xt_d = nc.dram_tensor("xt", [E, S], f16, kind="ExternalInput")        # x[b].T
    wkqv_d = nc.dram_tensor("wkqv", [E, 3 * A * H], f16, kind="ExternalInput")
    wo_d = nc.dram_tensor("wo", [A * H, E], f16, kind="ExternalInput")    # W_O.T
    m1_d = nc.dram_tensor("m1", [P, S + P], f16, kind="ExternalInput")
    id_d = nc.dram_tensor("ident", [P, P], f16, kind="ExternalInput")
    out_d = nc.dram_tensor("out", [S, E], f16, kind="ExternalOutput")

    ET = E // P       # 8 e-tiles
    CT = S // P       # 8 c-tiles
    NC = S // 512     # 2 512-chunks

    with TileContext(nc) as tc:
        with (
            tc.tile_pool(name="inp", bufs=1) as inp,
            tc.tile_pool(name="kqv", bufs=1) as kqv,
            tc.tile_pool(name="epool", bufs=4) as epool,
            tc.tile_pool(name="stats", bufs=12) as stats,
            tc.tile_pool(name="outp", bufs=3) as outp,
            tc.tile_pool(name="psq", bufs=4, space="PSUM") as psq,
            tc.tile_pool(name="pss", bufs=2, space="PSUM") as pss,
        ):
            # ---- SBUF destinations ----
            xT = inp.tile([P, ET, S], f16, tag="xT")
            wkqv = inp.tile([P, ET, 3 * A * H], f16, tag="wkqv")
            wo = inp.tile([P, ET, E], f16, tag="wo")
            m1 = inp.tile([P, S + P], f16, tag="m1")
            ident = inp.tile([P, P], f16, tag="ident")

            # ---- loads, ordered so first-needed data lands first ----
            nc.sync.dma_start(ident[:], id_d[:])
            nc.sync.dma_start(m1[:], m1_d[:])
            for t in range(ET):   # pair-0 K columns (first kq group)
                nc.sync.dma_start(wkqv[:, t, 0:P], wkqv_d[t * P:(t + 1) * P, 0:P])
            for t in range(ET):   # first x half
                nc.sync.dma_start(xT[:, t, 0:512], xt_d[t * P:(t + 1) * P, 0:512])
            for t in range(ET):   # pair-0 Q columns
                nc.sync.dma_start(wkqv[:, t, A * H:A * H + P],
                                  wkqv_d[t * P:(t + 1) * P, A * H:A * H + P])
            for t in range(ET):   # second x half
                nc.sync.dma_start(xT[:, t, 512:S], xt_d[t * P:(t + 1) * P, 512:S])
            for t in range(ET):   # K weights, rest
                nc.sync.dma_start(wkqv[:, t, P:A * H], wkqv_d[t * P:(t + 1) * P, P:A * H])
            for t in range(ET):   # Q weights, rest
                nc.sync.dma_start(wkqv[:, t, A * H + P:2 * A * H],
                                  wkqv_d[t * P:(t + 1) * P, A * H + P:2 * A * H])
            for t in range(ET):   # V weights
                nc.sync.dma_start(wkqv[:, t, 2 * A * H:3 * A * H],
                                  wkqv_d[t * P:(t + 1) * P, 2 * A * H:3 * A * H])
            for t in range(ET):   # output projection weights (needed last)
                nc.sync.dma_start(wo[:, t, :], wo_d[t * P:(t + 1) * P, :])

            K_T = kqv.tile([P, A // 2, S], f16, tag="K_T")   # pair-stacked [2h, c]
            Q_T = kqv.tile([P, A // 2, S], f16, tag="Q_T")
            V = kqv.tile([P, CT, A * H], f16, tag="V")       # [c, f]
            zT = kqv.tile([P, A // 2, S], f16, tag="zT")     # pair-stacked [f, C]

            # ---- PE warm-up: ride out the HAM throttle during the DMA wait ----
            wps = psq.tile([P, 512], f32, tag="psq", name="warm")
            for w in range(38):
                nc.tensor.matmul(wps[:, :P], ident[:], ident[:],
                                 start=(w == 0), stop=(w == 37),
                                 skip_group_check=True)
            wsb = stats.tile([P, 1], f32, tag="ssum", name="warmsink")
            nc.vector.reduce_max(wsb[:], wps[:, :P], axis=mybir.AxisListType.X)

            # ---- dense projection groups (used as attention fillers) ----
            def kq_group(p, mat, cc):
                dst = K_T if mat == 0 else Q_T
                ps = psq.tile([P, 512], f32, tag="psq", name=f"q{p}{mat}{cc}")
                for et in range(ET):
                    nc.tensor.matmul(
                        ps[:],
                        wkqv[:, et, mat * A * H + p * P: mat * A * H + (p + 1) * P],
                        xT[:, et, cc * 512:(cc + 1) * 512],
                        start=(et == 0), stop=(et == ET - 1),
                    )
                nc.vector.tensor_copy(out=dst[:, p, cc * 512:(cc + 1) * 512], in_=ps[:])

            def v_group(fc, i):
                ps = psq.tile([P, 512], f32, tag="psq", name=f"v{fc}{i}")
                for et in range(ET):
                    nc.tensor.matmul(
                        ps[:],
                        xT[:, et, i * P:(i + 1) * P],
                        wkqv[:, et, 2 * A * H + fc * 512: 2 * A * H + (fc + 1) * 512],
                        start=(et == 0), stop=(et == ET - 1),
                    )
                nc.vector.tensor_copy(out=V[:, i, fc * 512:(fc + 1) * 512], in_=ps[:])

            def kq_pair_closures(p):
                return [lambda p=p, mat=mat, cc=cc: kq_group(p, mat, cc)
                        for cc in range(NC) for mat in (0, 1)]

            def v_closures(fc):
                return [lambda fc=fc, i=i: v_group(fc, i) for i in range(CT)]

            # ---- attention ----
            def attn_rows(p, fillers):
                """Scores+exp+mask/rowsum rows of pair p, interleaving filler
                closures between rows. Returns (heads, E tiles)."""
                heads = [(2 * p, 0), (2 * p + 1, H)]
                Ets = [epool.tile([P, EW], f16, tag="E", name=f"E{k}_{p}")
                       for k in range(2)]
                sums = [stats.tile([P, CT], f32, tag="ssum", name=f"sm{k}_{p}")
                        for k in range(2)]
                rcps = [stats.tile([P, CT], f32, tag="rcp", name=f"rc{k}_{p}")
                        for k in range(2)]
                fq = list(fillers)
                # late rows have the longest exp, so weight fillers there
                w = [2, 2, 2, 2, 3, 3, 3, 3]
                tot = sum(w)
                share = [max(0, round(len(fq) * wi / tot)) if fq else 0 for wi in w]
                fi = 0
                for i in range(CT):
                    n_i = i // 4 + 1
                    vw = (i + 1) * P          # causally-valid row width
                    if i < 4:   # short rows fit a 512-wide psq slot; using the
                        # other pool splits the exp-release chain between rows
                        rows = [psq.tile([P, 512], f32, tag="psq", name=f"r{k}_{i}")
                                for k in range(2)]
                    else:
                        rows = [pss.tile([P, 1024], f32, tag="srow", name=f"r{k}_{i}")
                                for k in range(2)]
                    for j in range(n_i):
                        ntrim = min(512, vw - j * 512)
                        for k, (a, off) in enumerate(heads):
                            nc.tensor.matmul(
                                rows[k][:, j * 512:j * 512 + ntrim],
                                K_T[off:off + H, p, i * P:(i + 1) * P],
                                Q_T[off:off + H, p, j * 512:j * 512 + ntrim],
                                start=True, stop=True,
                                skip_group_check=True,
                            )
                    for k, (a, off) in enumerate(heads):
                        nc.scalar.activation(
                            Ets[k][:, _off(i):_off(i) + vw], rows[k][:, :vw], Exp,
                        )
                        # fused: mask diagonal block (multiplicative 0/1) and
                        # produce the causal row sum, all in one DVE pass
                        nc.vector.scalar_tensor_tensor(
                            out=Ets[k][:, _off(i):_off(i) + vw],
                            in0=Ets[k][:, _off(i):_off(i) + vw],
                            scalar=1.0,
                            in1=m1[:, S - i * P:S - i * P + vw],
                            op0=mult,
                            op1=mult,
                            accum_out=sums[k][:, i:i + 1],
                        )
                    for _ in range(share[i]):
                        if fi < len(fq):
                            fq[fi]()
                            fi += 1
                while fi < len(fq):
                    fq[fi]()
                    fi += 1
                # normalization: batched reciprocal per head, V rows scaled on
                # gpsimd (descending i to match the AV consumption order)
                for k in range(2):
                    nc.vector.reciprocal(rcps[k][:], sums[k][:])
                for i in range(CT - 1, -1, -1):
                    for k, (a, off) in enumerate(heads):
                        nc.gpsimd.tensor_scalar_mul(
                            V[:, i, a * H:(a + 1) * H],
                            V[:, i, a * H:(a + 1) * H],
                            rcps[k][:, i:i + 1],
                        )
                return heads, Ets

            def av_closures(p, heads, Ets):
                """AV block of pair p as filler closures (descending i, causal
                N-trim, two heads col-split in one PSUM bank per chunk j)."""
                state = {}
                cs = []

                def step(j, i):
                    if i == CT - 1:
                        state[j] = psq.tile([P, 512], f32, tag="psq",
                                            name=f"za_{p}_{j}")
                    za = state[j]
                    ntrim = min(512, (i - 4 * j) * P + P)
                    for k, (a, off) in enumerate(heads):
                        nc.tensor.matmul(
                            za[off:off + H, :ntrim],
                            V[:, i, a * H:(a + 1) * H],
                            Ets[k][:, _off(i) + j * 512:_off(i) + j * 512 + ntrim],
                            start=(i == CT - 1), stop=(i == 4 * j),
                            skip_group_check=True,
                        )

                def copy(j):
                    nc.vector.tensor_copy(out=zT[:, p, j * 512:(j + 1) * 512],
                                          in_=state[j][:])

                for j in range(NC):
                    for i in range(CT - 1, 4 * j - 1, -1):
                        cs.append(lambda j=j, i=i: step(j, i))
                    cs.append(lambda j=j: copy(j))
                return cs

            # ---- merged schedule ----
            for pp, mat, cc in ((0, 0, 0), (0, 1, 0), (0, 0, 1), (0, 1, 1)):
                kq_group(pp, mat, cc)

            pair_fillers = {
                0: kq_pair_closures(1) + v_closures(0),
                1: kq_pair_closures(2),
                2: kq_pair_closures(3) + v_closures(1)[0:2],
                3: kq_pair_closures(4) + v_closures(1)[2:6],
                4: kq_pair_closures(5) + v_closures(1)[6:8],
                5: kq_pair_closures(6),
                6: kq_pair_closures(7),
                7: [],
            }
            av_prev = None
            for p in range(8):
                fillers = pair_fillers.get(p, [])
                if av_prev is not None:
                    fillers = av_prev + fillers
                heads, Ets = attn_rows(p, fillers)
                av_prev = av_closures(p, heads, Ets)
            for cl in av_prev:             # AV of pair 7
                cl()

            # ---- output projection ----
            for m in range(CT):
                for n_ in range(NC):
                    ps = psq.tile([P, 512], f32, tag="psq", name=f"o{m}{n_}")
                    for p2 in range(ET):
                        nc.tensor.matmul(
                            ps[:],
                            zT[:, p2, m * P:(m + 1) * P],
                            wo[:, p2, n_ * 512:(n_ + 1) * 512],
                            start=(p2 == 0), stop=(p2 == ET - 1),
                        )
                    ot = outp.tile([P, 512], f16, tag="ot")
                    last = m == CT - 1
                    # alternate cast engines; the exp stream is done by now so
                    # the scalar engine is free
                    if (m + n_) % 2 == 0 or last:
                        nc.scalar.copy(out=ot[:], in_=ps[:])
                    else:
                        nc.vector.tensor_copy(out=ot[:], in_=ps[:])
                    nq = 4 if last else 2
                    wq = 512 // nq
                    for hh in range(nq):
                        nc.sync.dma_start(
                            out_d[m * P:(m + 1) * P,
                                  n_ * 512 + hh * wq:n_ * 512 + (hh + 1) * wq],
                            ot[:, hh * wq:(hh + 1) * wq],
                        )

    # HW allows only one sync-wait per instruction (matmuls especially);
    # split excess waits into InstEventSemaphore like the bacc layer does.
    import bass_rust
    bass_rust.generate_event_semaphores(nc)
    return nc


def _host_prep(x, W_K, W_Q, W_V, W_O):
    """Pack per-core input dicts (host-side layout prep, fp16 casts)."""
    wk = W_K.transpose(2, 0, 1).reshape(E, A * H)
    wq = (W_Q / np.sqrt(H)).transpose(2, 0, 1).reshape(E, A * H)
    wv = W_V.transpose(2, 0, 1).reshape(E, A * H)
    wkqv = np.concatenate([wk, wq, wv], axis=1).astype(np.float16)
    wo = np.ascontiguousarray(W_O.T).astype(np.float16)

    r = np.arange(P)[:, None]
    d = np.arange(P)[None, :]
    # ones over the prefix | inclusive lower-triangular 0/1 diagonal block
    m1 = np.concatenate(
        [np.ones((P, S), dtype=np.float16),
         np.where(d <= r, 1.0, 0.0).astype(np.float16)], axis=1)
    ident = np.eye(P, dtype=np.float16)

    in_maps = []
    for b in range(B):
        in_maps.append({
            "xt": np.ascontiguousarray(x[b].T).astype(np.float16),
            "wkqv": wkqv,
            "wo": wo,
            "m1": m1,
            "ident": ident,
        })
    return in_maps


def _run(x, W_K, W_Q, W_V, W_O, **spmd_kwargs):
    from concourse.bass_utils import run_bass_kernel_spmd

    if "nc" not in _cache:
        _cache["nc"] = _build_nc()
    in_maps = _host_prep(
        np.asarray(x, dtype=np.float32), np.asarray(W_K, dtype=np.float32),
        np.asarray(W_Q, dtype=np.float32), np.asarray(W_V, dtype=np.float32),
        np.asarray(W_O, dtype=np.float32),
    )
    res = run_bass_kernel_spmd(_cache["nc"], in_maps, core_ids=list(range(B)),
                               **spmd_kwargs)
    out = np.stack([r["out"] for r in res.results], axis=0).astype(np.float32)
    return out, res


def kernel(x, W_K, W_Q, W_V, W_O):
    out, _ = _run(x, W_K, W_Q, W_V, W_O)
    return out


# revision 8
# speedup vs baseline: 1.5689x; 1.5689x over previous
"""Trainium2 Bass kernel for 16-head causal attention (transposed-softmax variant).

Problem shapes: x [8, 1024, 1024]; W_K/W_Q/W_V [16, 64, 1024]; W_O [1024, 1024].
Sharding: pure data-parallel over batch (8 batch elements -> 8 cores), weights
replicated, no collectives.

Per-core pipeline (one batch element, seq=1024, d_embed=1024, 16 heads x 64):
  1. QKV projections as K_T/Q_T [heads*64, seq] and V [seq, heads*64], fp16
     operands, fp32 PSUM accumulation. W_Q is pre-scaled by 1/sqrt(d_head) on
     the host so scores come out pre-scaled.
  2. Per head pair: scores S[c, C] = K[c].Q[C] for causal-allowed C-chunks
     only, the two heads' K=64 matmuls interleaved so they run concurrently in
     disjoint PE row-groups. No additive masking on the PE: the diagonal
     128-block is masked multiplicatively after exp (step 3).
  3. Softmax over C without max-subtraction (scores ~ N(0,1)): exp on the
     scalar engine (no accumulator), then ONE fused vector-engine
     tensor_tensor_reduce per (head, row-tile): E *= m1-slice (ones over the
     strict prefix, lower-triangular 0/1 over the diagonal block) in place,
     with accum_out = causal row sum. This keeps the scalar engine exp-only
     (no READ_ACCUMULATOR) and removes all mask matmuls from the PE.
     Reciprocals are batched per head ([128, 8] in one op); V rows are scaled
     by 1/rowsum on the gpsimd engine (otherwise idle).
  4. Z^T[h, C] += V'^T E per c-tile (descending i, N trimmed to the causal
     prefix, two heads col-split in one PSUM bank); output projection
     Z_flat @ W_O^T, PSUM->SBUF casts alternating vector/scalar engines.

Scheduling: the scalar engine carries ~110us of exp and the PE ~140us of
matmul; dense projection groups are spread as fillers across ALL eight pairs
(not front-loaded) so the PE never starves mid-kernel and the HAM clock gate
stays at full rate. Each pair's AV block interleaves into the next pair's
rows; kq-group weights for pair p+1 are emitted inside pair p.
"""

import numpy as np

S, E, A, H, B = 1024, 1024, 16, 64, 8
P = 128          # partitions

_cache = {}


def _off(i):
    """Compact E-buffer offset of row-tile i (valid width of row i is (i+1)*P)."""
    return P * i * (i + 1) // 2


EW = _off(8)     # 4608 columns total


def _build_nc():
    import concourse.bass as bass
    import concourse.mybir as mybir
    from concourse.tile import TileContext

    f16 = mybir.dt.float16
    f32 = mybir.dt.float32
    Exp = mybir.ActivationFunctionType.Exp
    mult = mybir.AluOpType.mult
    add = mybir.AluOpType.add

    nc = bass.Bass()
    xt_d = nc.dram_tensor("xt", [E, S], f16, kind="ExternalInput")        # x[b].T
    wkqv_d = nc.dram_tensor("wkqv", [E, 3 * A * H], f16, kind="ExternalInput")
    wo_d = nc.dram_tensor("wo", [A * H, E], f16, kind="ExternalInput")    # W_O.T
    m1_d = nc.dram_tensor("m1", [P, S + P], f16, kind="ExternalInput")
    id_d = nc.dram_tensor("ident", [P, P], f16, kind="ExternalInput")
    out_d = nc.dram_tensor("out", [S, E], f16, kind="ExternalOutput")

    ET = E // P       # 8 e-tiles
    CT = S // P       # 8 c-tiles
    NC = S // 512     # 2 512-chunks

    with TileContext(nc) as tc:
        with (
            tc.tile_pool(name="inp", bufs=1) as inp,
            tc.tile_pool(name="kqv", bufs=1) as kqv,
            tc.tile_pool(name="epool", bufs=4) as epool,
            tc.tile_pool(name="stats", bufs=12) as stats,
            tc.tile_pool(name="outp", bufs=3) as outp,
            tc.tile_pool(name="psq", bufs=4, space="PSUM") as psq,
            tc.tile_pool(name="pss", bufs=2, space="PSUM") as pss,
        ):
            # ---- SBUF destinations ----
            xT = inp.tile([P, ET, S], f16, tag="xT")
            wkqv = inp.tile([P, ET, 3 * A * H], f16, tag="wkqv")
            wo = inp.tile([P, ET, E], f16, tag="wo")
            m1 = inp.tile([P, S + P], f16, tag="m1")
            ident = inp.tile([P, P], f16, tag="ident")

            # ---- loads, ordered so first-needed data lands first ----
            nc.sync.dma_start(ident[:], id_d[:])
            nc.sync.dma_start(m1[:], m1_d[:])
            for t in range(ET):   # pair-0 K columns (first kq group)
                nc.sync.dma_start(wkqv[:, t, 0:P], wkqv_d[t * P:(t + 1) * P, 0:P])
            for t in range(ET):   # first x half
                nc.sync.dma_start(xT[:, t, 0:512], xt_d[t * P:(t + 1) * P, 0:512])
            for t in range(ET):   # pair-0 Q columns
                nc.sync.dma_start(wkqv[:, t, A * H:A * H + P],
                                  wkqv_d[t * P:(t + 1) * P, A * H:A * H + P])
            for t in range(ET):   # second x half
                nc.sync.dma_start(xT[:, t, 512:S], xt_d[t * P:(t + 1) * P, 512:S])
            for t in range(ET):   # K weights, rest
                nc.sync.dma_start(wkqv[:, t, P:A * H], wkqv_d[t * P:(t + 1) * P, P:A * H])
            for t in range(ET):   # Q weights, rest
                nc.sync.dma_start(wkqv[:, t, A * H + P:2 * A * H],
                                  wkqv_d[t * P:(t + 1) * P, A * H + P:2 * A * H])
            for t in range(ET):   # V weights
                nc.sync.dma_start(wkqv[:, t, 2 * A * H:3 * A * H],
                                  wkqv_d[t * P:(t + 1) * P, 2 * A * H:3 * A * H])
            for t in range(ET):   # output projection weights (needed last)
                nc.sync.dma_start(wo[:, t, :], wo_d[t * P:(t + 1) * P, :])

            K_T = kqv.tile([P, A // 2, S], f16, tag="K_T")   # pair-stacked [2h, c]
            Q_T = kqv.tile([P, A // 2, S], f16, tag="Q_T")
            V = kqv.tile([P, CT, A * H], f16, tag="V")       # [c, f]
            zT = kqv.tile([P, A // 2, S], f16, tag="zT")     # pair-stacked [f, C]

            # ---- PE warm-up: ride out the HAM throttle during the DMA wait ----
            wps = psq.tile([P, 512], f32, tag="psq", name="warm")
            for w in range(38):
                nc.tensor.matmul(wps[:, :P], ident[:], ident[:],
                                 start=(w == 0), stop=(w == 37),
                                 skip_group_check=True)
            wsb = stats.tile([P, 1], f32, tag="ssum", name="warmsink")
            nc.vector.reduce_max(wsb[:], wps[:, :P], axis=mybir.AxisListType.X)
            # perf probe: tensor_scalar with accumulator on a 1024-wide fp16
            # row — its trace duration reveals whether accum_out keeps the
            # 4x perf mode (~330ns) or drops to 1x (~1.1us)
            wacc = stats.tile([P, 1], f32, tag="ssum", name="waccprobe")
            nc.vector.tensor_scalar(
                out=m1[:, 0:S], in0=m1[:, 0:S], scalar1=1.0, scalar2=0.0,
                op0=mult, op1=add, accum_out=wacc[:])

            # ---- dense projection groups (used as attention fillers) ----
            def kq_group(p, mat, cc):
                dst = K_T if mat == 0 else Q_T
                ps = psq.tile([P, 512], f32, tag="psq", name=f"q{p}{mat}{cc}")
                for et in range(ET):
                    nc.tensor.matmul(
                        ps[:],
                        wkqv[:, et, mat * A * H + p * P: mat * A * H + (p + 1) * P],
                        xT[:, et, cc * 512:(cc + 1) * 512],
                        start=(et == 0), stop=(et == ET - 1),
                    )
                # kq casts ride the scalar engine: exp leaves it ~20% idle,
                # while the vector engine carries the mask/rowsum pass
                nc.scalar.copy(out=dst[:, p, cc * 512:(cc + 1) * 512], in_=ps[:])

            def v_group(fc, i):
                ps = psq.tile([P, 512], f32, tag="psq", name=f"v{fc}{i}")
                for et in range(ET):
                    nc.tensor.matmul(
                        ps[:],
                        xT[:, et, i * P:(i + 1) * P],
                        wkqv[:, et, 2 * A * H + fc * 512: 2 * A * H + (fc + 1) * 512],
                        start=(et == 0), stop=(et == ET - 1),
                    )
                nc.vector.tensor_copy(out=V[:, i, fc * 512:(fc + 1) * 512], in_=ps[:])

            def kq_pair_closures(p):
                return [lambda p=p, mat=mat, cc=cc: kq_group(p, mat, cc)
                        for cc in range(NC) for mat in (0, 1)]

            def v_closures(fc):
                return [lambda fc=fc, i=i: v_group(fc, i) for i in range(CT)]

            # ---- attention ----
            def attn_rows(p, fillers):
                """Scores+exp+mask/rowsum rows of pair p, interleaving filler
                closures between rows. Returns (heads, E tiles)."""
                heads = [(2 * p, 0), (2 * p + 1, H)]
                Ets = [epool.tile([P, EW], f16, tag="E", name=f"E{k}_{p}")
                       for k in range(2)]
                sums = [stats.tile([P, CT], f32, tag="ssum", name=f"sm{k}_{p}")
                        for k in range(2)]
                rcps = [stats.tile([P, CT], f32, tag="rcp", name=f"rc{k}_{p}")
                        for k in range(2)]
                fq = list(fillers)
                # late rows have the longest exp, so weight fillers there
                w = [2, 2, 2, 2, 3, 3, 3, 3]
                tot = sum(w)
                share = [max(0, round(len(fq) * wi / tot)) if fq else 0 for wi in w]
                fi = 0
                for i in range(CT):
                    n_i = i // 4 + 1
                    vw = (i + 1) * P          # causally-valid row width
                    if i < 4:   # short rows fit a 512-wide psq slot; using the
                        # other pool splits the exp-release chain between rows
                        rows = [psq.tile([P, 512], f32, tag="psq", name=f"r{k}_{i}")
                                for k in range(2)]
                    else:
                        rows = [pss.tile([P, 1024], f32, tag="srow", name=f"r{k}_{i}")
                                for k in range(2)]
                    for j in range(n_i):
                        ntrim = min(512, vw - j * 512)
                        for k, (a, off) in enumerate(heads):
                            nc.tensor.matmul(
                                rows[k][:, j * 512:j * 512 + ntrim],
                                K_T[off:off + H, p, i * P:(i + 1) * P],
                                Q_T[off:off + H, p, j * 512:j * 512 + ntrim],
                                start=True, stop=True,
                                skip_group_check=True,
                            )
                    for k, (a, off) in enumerate(heads):
                        nc.scalar.activation(
                            Ets[k][:, _off(i):_off(i) + vw], rows[k][:, :vw], Exp,
                        )
                        # fused: mask diagonal block (multiplicative 0/1) and
                        # produce the causal row sum, all in one DVE pass
                        nc.vector.scalar_tensor_tensor(
                            out=Ets[k][:, _off(i):_off(i) + vw],
                            in0=Ets[k][:, _off(i):_off(i) + vw],
                            scalar=1.0,
                            in1=m1[:, S - i * P:S - i * P + vw],
                            op0=mult,
                            op1=mult,
                            accum_out=sums[k][:, i:i + 1],
                        )
                    for _ in range(share[i]):
                        if fi < len(fq):
                            fq[fi]()
                            fi += 1
                while fi < len(fq):
                    fq[fi]()
                    fi += 1
                # normalization: batched reciprocal per head, then ONE
                # broadcast tensor_tensor per head scales all 8 V row-tiles
                for k, (a, off) in enumerate(heads):
                    nc.vector.reciprocal(rcps[k][:], sums[k][:])
                    nc.vector.tensor_tensor(
                        out=V[:, 0:CT, a * H:(a + 1) * H],
                        in0=V[:, 0:CT, a * H:(a + 1) * H],
                        in1=rcps[k][:, 0:CT, None].to_broadcast([P, CT, H]),
                        op=mult,
                    )
                return heads, Ets

            def av_closures(p, heads, Ets):
                """AV block of pair p as filler closures (descending i, causal
                N-trim, two heads col-split in one PSUM bank per chunk j)."""
                state = {}
                cs = []

                def step(j, i):
                    if i == CT - 1:
                        state[j] = psq.tile([P, 512], f32, tag="psq",
                                            name=f"za_{p}_{j}")
                    za = state[j]
                    ntrim = min(512, (i - 4 * j) * P + P)
                    for k, (a, off) in enumerate(heads):
                        nc.tensor.matmul(
                            za[off:off + H, :ntrim],
                            V[:, i, a * H:(a + 1) * H],
                            Ets[k][:, _off(i) + j * 512:_off(i) + j * 512 + ntrim],
                            start=(i == CT - 1), stop=(i == 4 * j),
                            skip_group_check=True,
                        )

                def copy(j):
                    nc.vector.tensor_copy(out=zT[:, p, j * 512:(j + 1) * 512],
                                          in_=state[j][:])

                for j in range(NC):
                    for i in range(CT - 1, 4 * j - 1, -1):
                        cs.append(lambda j=j, i=i: step(j, i))
                    cs.append(lambda j=j: copy(j))
                return cs

            # ---- merged schedule ----
            for pp, mat, cc in ((0, 0, 0), (0, 1, 0), (0, 0, 1), (0, 1, 1)):
                kq_group(pp, mat, cc)

            pair_fillers = {
                0: kq_pair_closures(1) + v_closures(0),
                1: kq_pair_closures(2),
                2: kq_pair_closures(3) + v_closures(1)[0:2],
                3: kq_pair_closures(4) + v_closures(1)[2:6],
                4: kq_pair_closures(5) + v_closures(1)[6:8],
                5: kq_pair_closures(6),
                6: kq_pair_closures(7),
                7: [],
            }
            av_prev = None
            for p in range(8):
                fillers = pair_fillers.get(p, [])
                if av_prev is not None:
                    fillers = av_prev + fillers
                heads, Ets = attn_rows(p, fillers)
                av_prev = av_closures(p, heads, Ets)
            for cl in av_prev:             # AV of pair 7
                cl()

            # ---- output projection ----
            for m in range(CT):
                for n_ in range(NC):
                    ps = psq.tile([P, 512], f32, tag="psq", name=f"o{m}{n_}")
                    for p2 in range(ET):
                        nc.tensor.matmul(
                            ps[:],
                            zT[:, p2, m * P:(m + 1) * P],
                            wo[:, p2, n_ * 512:(n_ + 1) * 512],
                            start=(p2 == 0), stop=(p2 == ET - 1),
                        )
                    ot = outp.tile([P, 512], f16, tag="ot")
                    last = m == CT - 1
                    # alternate cast engines; the exp stream is done by now so
                    # the scalar engine is free
                    if (m + n_) % 2 == 0 or last:
                        nc.scalar.copy(out=ot[:], in_=ps[:])
                    else:
                        nc.vector.tensor_copy(out=ot[:], in_=ps[:])
                    nq = 4 if last else 2
                    wq = 512 // nq
                    for hh in range(nq):
                        nc.sync.dma_start(
                            out_d[m * P:(m + 1) * P,
                                  n_ * 512 + hh * wq:n_ * 512 + (hh + 1) * wq],
                            ot[:, hh * wq:(hh + 1) * wq],
                        )

    # HW allows only one sync-wait per instruction (matmuls especially);
    # split excess waits into InstEventSemaphore like the bacc layer does.
    import bass_rust
    bass_rust.generate_event_semaphores(nc)
    return nc


def _host_prep(x, W_K, W_Q, W_V, W_O):
    """Pack per-core input dicts (host-side layout prep, fp16 casts)."""
    wk = W_K.transpose(2, 0, 1).reshape(E, A * H)
    wq = (W_Q / np.sqrt(H)).transpose(2, 0, 1).reshape(E, A * H)
    wv = W_V.transpose(2, 0, 1).reshape(E, A * H)
    wkqv = np.concatenate([wk, wq, wv], axis=1).astype(np.float16)
    wo = np.ascontiguousarray(W_O.T).astype(np.float16)

    r = np.arange(P)[:, None]
    d = np.arange(P)[None, :]
    # ones over the prefix | inclusive lower-triangular 0/1 diagonal block
    m1 = np.concatenate(
        [np.ones((P, S), dtype=np.float16),
         np.where(d <= r, 1.0, 0.0).astype(np.float16)], axis=1)
    ident = np.eye(P, dtype=np.float16)

    in_maps = []
    for b in range(B):
        in_maps.append({
            "xt": np.ascontiguousarray(x[b].T).astype(np.float16),
            "wkqv": wkqv,
            "wo": wo,
            "m1": m1,
            "ident": ident,
        })
    return in_maps


def _run(x, W_K, W_Q, W_V, W_O, **spmd_kwargs):
    from concourse.bass_utils import run_bass_kernel_spmd

    if "nc" not in _cache:
        _cache["nc"] = _build_nc()
    in_maps = _host_prep(
        np.asarray(x, dtype=np.float32), np.asarray(W_K, dtype=np.float32),
        np.asarray(W_Q, dtype=np.float32), np.asarray(W_V, dtype=np.float32),
        np.asarray(W_O, dtype=np.float32),
    )
    res = run_bass_kernel_spmd(_cache["nc"], in_maps, core_ids=list(range(B)),
                               **spmd_kwargs)
    out = np.stack([r["out"] for r in res.results], axis=0).astype(np.float32)
    return out, res


def kernel(x, W_K, W_Q, W_V, W_O):
    out, _ = _run(x, W_K, W_Q, W_V, W_O)
    return out
